# revision 3
# baseline (speedup 1.0000x reference)
"""Diag-scale kernel: out = input * W (input @ diag(W)).

input: (16384, 4096) f32, W: (4096,) f32. Data-parallel over 8 NeuronCores:
each core gets 2048 rows. W is replicated host-side to [128, 4096] so the
kernel loads it with one plain DMA (~6us) instead of a ~35us gpsimd
partition_broadcast that serialized the whole store stream behind it.
Memory-bound: each core streams 32 MiB in and 32 MiB out; 2 MiB chunks
(one row per SBUF partition) keep loads, muls and stores pipelined from
~15us onward so the HBM domain stays saturated in both directions.
"""

import os
import numpy as np

import concourse.bacc as bacc
import concourse.mybir as mybir
from concourse.tile import TileContext
from concourse.bass_utils import run_bass_kernel_spmd

N = 16384
D = 4096
NCORES = 8
ROWS = N // NCORES          # 2048 rows per core
P = 128                     # SBUF partitions
NCHUNK = ROWS // P          # 16 chunks of [128, 4096] = 2 MiB each
IO_BUFS = 8                 # 8 x 16KB/partition slots + 16KB W = 144KB/partition

last_exec_time_ns = None
last_trace_dir = None
_built_nc = None


def _build():
    nc = bacc.Bacc(None, target_bir_lowering=False, debug=False)
    inp = nc.declare_dram_parameter("input", [ROWS, D], mybir.dt.float32, isOutput=False)
    w = nc.declare_dram_parameter("w", [P, D], mybir.dt.float32, isOutput=False)
    out = nc.declare_dram_parameter("out", [ROWS, D], mybir.dt.float32, isOutput=True)

    with TileContext(nc) as tc:
        with (
            tc.tile_pool(name="wpool", bufs=1) as wpool,
            tc.tile_pool(name="io", bufs=IO_BUFS) as io,
        ):
            wt = wpool.tile([P, D], mybir.dt.float32)
            # 2 MiB pre-replicated W rides the SWDGE queue, which is
            # otherwise idle until the first store — both HWDGE rings start
            # streaming input immediately.
            nc.gpsimd.dma_start(out=wt[:], in_=w[:, :])

            # (row_start, col_start, ncols); last two 2 MiB chunks split
            # into column halves so the final mul+store drain is short.
            chunks = [(P * k, 0, D) for k in range(NCHUNK - 2)]
            for k in (NCHUNK - 2, NCHUNK - 1):
                chunks += [(P * k, 0, D // 2), (P * k, D // 2, D // 2)]

            for idx, (rs, c0, ncols) in enumerate(chunks):
                t = io.tile([P, ncols], mybir.dt.float32)
                # Loads alternate between the two HWDGE rings (SP and ACT);
                # stores go via SWDGE (gpsimd) so load and store completions
                # ride independent queues and the drain never blocks loads.
                ldeng = nc.sync if idx % 2 == 0 else nc.scalar
                ldeng.dma_start(out=t[:], in_=inp[rs : rs + P, c0 : c0 + ncols])
                nc.vector.tensor_mul(
                    out=t[:], in0=t[:], in1=wt[:, c0 : c0 + ncols]
                )
                nc.gpsimd.dma_start(out=out[rs : rs + P, c0 : c0 + ncols], in_=t[:])
    nc.compile()
    return nc


def kernel(input, W):
    global last_exec_time_ns, _built_nc
    input = np.ascontiguousarray(np.asarray(input, dtype=np.float32))
    W = np.asarray(W, dtype=np.float32).reshape(D)

    if _built_nc is None:
        _built_nc = _build()
    nc = _built_nc

    # W replicated across partitions host-side: the kernel loads it as one
    # plain [128, 4096] DMA instead of broadcasting on-chip.
    w_rep = np.ascontiguousarray(np.broadcast_to(W, (P, D)))
    shards = input.reshape(NCORES, ROWS, D)
    in_maps = [{"input": shards[c], "w": w_rep} for c in range(NCORES)]

    global last_trace_dir
    trace = os.environ.get("KERNEL_TRACE", "0") == "1"
    kwargs = {}
    if trace:
        import tempfile

        last_trace_dir = tempfile.mkdtemp(prefix="diag_trace_")
        kwargs = {"trace": True, "tmpdir": last_trace_dir}
    res = run_bass_kernel_spmd(nc, in_maps, core_ids=list(range(NCORES)), **kwargs)
    last_exec_time_ns = res.exec_time_ns

    out = np.concatenate([res.results[c]["out"] for c in range(NCORES)], axis=0)
    return out
